# revision 4
# baseline (speedup 1.0000x reference)
"""Trainium2 Bass kernel, v2: big-matmul formulation (72-contraction).

Layout: activations per x1-plane as [72, 50, 50] bf16 tiles; partition row
= 3*x4 + c (x4 in [0,24), c in [0,3)).  x2/x3 zero-padded 48->50 in free
dims; x4 conv edge handled by truncating the band in the weights (no pad
rows, no halo duplication, no fixups).

Each conv tap (d1,d2,d3) = ONE 72x72 matmul per x2-chunk: lhsT[r=3*x4i+ci,
col=3*x4o+co] = K4[co,ci,a,b,c,x4i-x4o+1] (banded over x4, all 24 x4 in one
matmul).  Stage A: 54 passes (2 input groups x 27 taps); stage B: 28 passes
(27 composed W2oW1 taps + Wd residual).  5 x2-chunks {10,10,10,10,8} per
pass for PSUM bank capacity; A uses banks 0-4, B banks 5-7 in two halves.

This targets the serialized tensor-engine regime measured on this part:
per-matmul cost fits ~9.1 ns x M(out cols) weight-load + 0.42 ns x N(free)
stream + ~42 ns; tile_position tiling and column-splitting were measured
and gain nothing here, so fewest-matmuls-with-widest-output wins:
~31.7k matmuls/core (~0.89 us each) vs ~95k in the 30x32-tile scheme.

Sharding: 8 cores = 2 batch x 4 x1-slabs of 12, recompute halo 8 per side
(28 local planes; layer l computes temp1 on [2l+1, 27-2l), out on
[2l+2, 26-2l)).  Layer outputs ping-pong through per-core DRAM in bf16.
"""

import numpy as np
import ml_dtypes

LAYERS = 4
B, C, D1, D2, D3, D4 = 2, 3, 48, 48, 48, 24
NCORES = 8
SLAB = 12
HALO = 8
NPLANES = SLAB + 2 * HALO   # 28
NROWS = 72                  # 3*x4 + c
PW = 50
CHUNK_ROWS = [10, 10, 10, 10, 8]
CHUNK_OFF = [0, 10, 20, 30, 40]
BH = [(0, (0, 1, 2)), (1, (3, 4))]  # stage-B PSUM halves (3 banks)

BF16 = ml_dtypes.bfloat16

PLANE_LIMIT = None

_cached = {}


def _band72(K4):
    """K4: [co, ci(3), 3,3,3, e(3)] -> [27, 72, 72] lhsT per (d1,d2,d3) tap.
    lhsT[tap, 3*x4i+ci, 3*x4o+co] = K4[co, ci, a, b, c, x4i-x4o+1]."""
    out = np.zeros((27, NROWS, NROWS), np.float32)
    x4o = np.arange(D4)
    for e in range(3):
        x4i = x4o + e - 1
        m = (x4i >= 0) & (x4i < D4)
        for pi in range(27):
            a, b_, c_ = pi // 9, (pi // 3) % 3, pi % 3
            for ci in range(3):
                for co in range(3):
                    out[pi, 3 * x4i[m] + ci, 3 * x4o[m] + co] = K4[co, ci, a, b_, c_, e]
    return out


def _build_weights(Wg, bg, W1, b1, W2, b2, Wd, bd):
    Wg = np.asarray(Wg, np.float32)
    W1 = np.asarray(W1, np.float32)
    W2 = np.asarray(W2, np.float32)
    Wd = np.asarray(Wd, np.float32)
    W21 = np.einsum("lome,lmiabc->loiabce", W2[:, :, :, 0, 0, 0, :], W1[..., 0])

    # stored row-major-first for contiguous SBUF DMA: [L, row(128), 2, 27, col]
    # rows 72..127 are zero: K is padded to 128 partitions; zero weight rows
    # null out whatever the padded rhs partitions contain, so pad content
    # only has to be finite (pool slots are pre-zeroed once).
    wA = np.zeros((LAYERS, 128, 2, 27, NROWS), np.float32)
    wB = np.zeros((LAYERS, 128, 27, NROWS), np.float32)
    wD = np.zeros((LAYERS, 128, NROWS), np.float32)
    for l in range(LAYERS):
        wA[l, :NROWS, 0] = _band72(Wg[l, :, 0:3]).transpose(1, 0, 2)
        wA[l, :NROWS, 1] = _band72(Wg[l, :, 3:6]).transpose(1, 0, 2)
        wB[l, :NROWS] = _band72(W21[l]).transpose(1, 0, 2)
        for x4 in range(D4):
            for ci in range(3):
                for co in range(3):
                    wD[l, 3 * x4 + ci, 3 * x4 + co] = Wd[l, co, ci, 0, 0, 0, 0]
    return {
        "wA": wA.astype(BF16),
        "wB": wB.astype(BF16),
        "wD": wD.astype(BF16),
    }


def _bias_tables(bg, b1, b2, bd, W2, q):
    """[L, 2, NPLANES, 72, 2] fp32: col 0 = mask, col 1 = bias.
    Zero on globally-invalid x1 planes (acts as conv zero-pad)."""
    bg = np.asarray(bg, np.float32)
    b1 = np.asarray(b1, np.float32)
    b2 = np.asarray(b2, np.float32)
    bd = np.asarray(bd, np.float32)
    W2 = np.asarray(W2, np.float32)
    tab = np.zeros((LAYERS, 2, NPLANES, NROWS, 2), np.float32)
    for l in range(LAYERS):
        rowA = np.zeros(NROWS, np.float32)
        rowB = np.zeros(NROWS, np.float32)
        for x4 in range(D4):
            for c in range(3):
                r = 3 * x4 + c
                rowA[r] = bg[l, c]
                acc = b2[l, c] + bd[l, c]
                for e in range(3):
                    if 0 <= x4 + e - 1 < D4:
                        acc += float(np.dot(W2[l, c, :, 0, 0, 0, e], b1[l]))
                rowB[r] = acc
        for p in range(NPLANES):
            g = 12 * q - HALO + p
            if 0 <= g < D1:
                tab[l, 0, p, :, 0] = 1.0
                tab[l, 0, p, :, 1] = rowA
                tab[l, 1, p, :, 0] = 1.0
                tab[l, 1, p, :, 1] = rowB
    return tab


def _make_slab(vol, q):
    """vol: [C, D1, D2, D3, D4] fp32 -> [72, NPLANES, 50, 50] bf16 for core q."""
    slab = np.zeros((NROWS, NPLANES, PW, PW), BF16)
    p = np.arange(NPLANES)
    g = 12 * q - HALO + p
    pmask = (g >= 0) & (g < D1)
    ps, gs = p[pmask], g[pmask]
    # row 3*x4+c <- vol[c, g, :, :, x4]: [D4*C, len(ps), 48, 48]
    vt = vol.transpose(4, 0, 1, 2, 3)  # [D4, C, D1, D2, D3]
    slab[:, ps, 1:49, 1:49] = vt[:, :, gs].reshape(
        NROWS, len(ps), 48, 48).astype(BF16)
    return slab


def _build_program():
    import concourse.bass as bass
    import concourse.mybir as mybir
    import concourse.tile as tile
    from concourse import bacc

    f32 = mybir.dt.float32
    bf16 = mybir.dt.bfloat16

    nc = bacc.Bacc("TRN2", target_bir_lowering=False, debug=False,
                   num_devices=NCORES)

    fsrc = nc.dram_tensor("fsrc", [NROWS, NPLANES, PW, PW], bf16, kind="ExternalInput")
    bndd = nc.dram_tensor("bndd", [NROWS, NPLANES, PW, PW], bf16, kind="ExternalInput")
    wAd = nc.dram_tensor("wAd", [LAYERS, 128, 2, 27, NROWS], bf16, kind="ExternalInput")
    wBd = nc.dram_tensor("wBd", [LAYERS, 128, 27, NROWS], bf16, kind="ExternalInput")
    wDd = nc.dram_tensor("wDd", [LAYERS, 128, NROWS], bf16, kind="ExternalInput")
    btd = nc.dram_tensor("btd", [LAYERS, 2, NPLANES, NROWS, 2], f32, kind="ExternalInput")
    bufA = nc.dram_tensor("bufA", [NROWS, NPLANES, PW, PW], bf16, kind="Internal")
    bufB = nc.dram_tensor("bufB", [NROWS, NPLANES, PW, PW], bf16, kind="Internal")
    outd = nc.dram_tensor("outd", [NROWS, SLAB, 48, 48], f32, kind="ExternalOutput")

    with tile.TileContext(nc) as tc:
        with (
            tc.tile_pool(name="wpool", bufs=2) as wpool,
            tc.tile_pool(name="spool", bufs=6) as spool,
            tc.tile_pool(name="bpool", bufs=5) as bpool,
            tc.tile_pool(name="tpool", bufs=4) as tpool,
            tc.tile_pool(name="opool", bufs=3) as opool,
            tc.tile_pool(name="fpool", bufs=2) as fpool,
            tc.tile_pool(name="btpool", bufs=4) as btpool,
            tc.tile_pool(name="ppa", bufs=1, space="PSUM") as ppa,
            tc.tile_pool(name="ppb", bufs=1, space="PSUM") as ppb,
        ):
            def zero_borders(t):
                nc.vector.memset(t[0:NROWS, 0, :], 0.0)
                nc.vector.memset(t[0:NROWS, PW - 1, :], 0.0)
                nc.vector.memset(t[0:NROWS, :, 0], 0.0)
                nc.vector.memset(t[0:NROWS, :, PW - 1], 0.0)

            COPY = mybir.ActivationFunctionType.Identity
            MUL = mybir.AluOpType.mult
            ADD = mybir.AluOpType.add

            def drain(eng_is_act, dst_ap, src_ap, mask_ap, bias_ap):
                if eng_is_act:
                    nc.scalar.activation(dst_ap, src_ap, COPY,
                                         bias=bias_ap, scale=mask_ap)
                else:
                    nc.vector.tensor_scalar(dst_ap, src_ap, mask_ap, bias_ap,
                                            MUL, ADD)

            # Pre-zero partition rows 72..127 of every pooled rhs slot once:
            # later allocations reuse these slots and only ever write rows
            # 0..71, so the pad rows stay zero (zero weights x finite pad
            # rows contribute nothing; this guards against NaN garbage).
            for pool, tag, nslots in ((spool, "sw", 6), (bpool, "bw", 5),
                                      (tpool, "tw", 4)):
                for i in range(nslots):
                    zt = pool.tile([128, PW, PW], bf16, name=f"z{tag}{i}",
                                   tag=tag)
                    # [64:128]: partition ranges must be aligned power-of-two
                    # blocks; rows 64..71 are rewritten by every real load.
                    nc.vector.memset(zt[64:128, :, :], 0.0)

            for l in range(LAYERS):
                src = [fsrc, bufA, bufB, bufA][l]
                dst = [bufA, bufB, bufA, None][l]
                wa = wpool.tile([128, 2, 27, NROWS], bf16, name=f"wa{l}", tag="wa")
                wb = wpool.tile([128, 27, NROWS], bf16, name=f"wb{l}", tag="wb")
                wd = wpool.tile([128, NROWS], bf16, name=f"wd{l}", tag="wd")
                nc.sync.dma_start(wa[:], wAd.ap()[l])
                nc.sync.dma_start(wb[:], wBd.ap()[l])
                nc.sync.dma_start(wd[:], wDd.ap()[l])

                A_lo, A_hi = 2 * l + 1, 27 - 2 * l
                B_lo, B_hi = 2 * l + 2, 26 - 2 * l
                if PLANE_LIMIT is not None:
                    A_lo, A_hi = max(A_lo, PLANE_LIMIT[0]), min(A_hi, PLANE_LIMIT[1])
                    B_lo, B_hi = max(B_lo, A_lo + 1), min(B_hi, A_hi - 1)
                scache, bcache, tcache = {}, {}, {}

                for x in range(A_lo, A_hi):
                    for p in (x - 1, x, x + 1):
                        if p not in scache:
                            st = spool.tile([128, PW, PW], bf16,
                                            name=f"s{l}_{p}", tag="sw")
                            nc.sync.dma_start(st[0:NROWS], src.ap()[:, p])
                            scache[p] = st
                        if p not in bcache:
                            bt_ = bpool.tile([128, PW, PW], bf16,
                                             name=f"b{l}_{p}", tag="bw")
                            nc.sync.dma_start(bt_[0:NROWS], bndd.ap()[:, p])
                            bcache[p] = bt_
                    bta = btpool.tile([NROWS, 2], f32, name=f"bta{l}_{x}", tag="bt")
                    nc.sync.dma_start(bta[:], btd.ap()[l, 0, x])

                    # ---- stage A: temp1 plane x; 54 passes x 5 chunks ----
                    pt = ppa.tile([NROWS, 5, 512], f32, name=f"pa{l}_{x}", tag="pa")
                    for pi in range(54):
                        g, p27 = divmod(pi, 27)
                        a, b_, c_ = p27 // 9, (p27 // 3) % 3, p27 % 3
                        rt = (scache if g == 0 else bcache)[x + a - 1]
                        for k in range(5):
                            nr = CHUNK_ROWS[k]
                            nc.tensor.matmul(
                                pt[:, k, :48 * nr],
                                wa[:, g, p27, :],
                                rt[0:128,
                                   CHUNK_OFF[k] + b_:CHUNK_OFF[k] + b_ + nr,
                                   c_:c_ + 48],
                                start=(pi == 0), stop=(pi == 53),
                                skip_group_check=True,
                            )
                    tt = tpool.tile([128, PW, PW], bf16, name=f"t{l}_{x}", tag="tw")
                    tcache[x] = tt
                    zero_borders(tt)
                    for k in range(5):
                        nr = CHUNK_ROWS[k]
                        drain(k in (0, 2, 4),
                              tt[0:NROWS, 1 + CHUNK_OFF[k]:1 + CHUNK_OFF[k] + nr,
                                 1:49],
                              pt[:, k, :48 * nr],
                              bta[:, 0:1], bta[:, 1:2])

                    # ---- stage B for plane y = x-1 ----
                    y = x - 1
                    if not (B_lo <= y < B_hi):
                        continue
                    final = l == LAYERS - 1
                    if final and not (HALO <= y < HALO + SLAB):
                        continue
                    btb = btpool.tile([NROWS, 2], f32, name=f"btb{l}_{y}", tag="bt")
                    nc.sync.dma_start(btb[:], btd.ap()[l, 1, y])
                    ot = ft = None
                    for half, ks in BH:
                        qt = ppb.tile([NROWS, 3, 512], f32, name=f"pb{l}_{y}_{half}",
                                      tag="pb")
                        for pi in range(28):
                            for k in ks:
                                nr = CHUNK_ROWS[k]
                                kr = k % 3
                                if pi < 27:
                                    a, b_, c_ = pi // 9, (pi // 3) % 3, pi % 3
                                    rt = tcache[y + a - 1]
                                    nc.tensor.matmul(
                                        qt[:, kr, :48 * nr],
                                        wb[:, pi, :],
                                        rt[0:128,
                                           CHUNK_OFF[k] + b_:CHUNK_OFF[k] + b_ + nr,
                                           c_:c_ + 48],
                                        start=(pi == 0), stop=False,
                                        skip_group_check=True,
                                    )
                                else:
                                    rt = scache[y]
                                    nc.tensor.matmul(
                                        qt[:, kr, :48 * nr],
                                        wd[:, :],
                                        rt[0:128,
                                           1 + CHUNK_OFF[k]:1 + CHUNK_OFF[k] + nr,
                                           1:49],
                                        start=False, stop=True,
                                        skip_group_check=True,
                                    )
                        if not final:
                            if half == 0:
                                ot = opool.tile([NROWS, PW, PW], bf16,
                                                name=f"o{l}_{y}", tag="ow")
                                zero_borders(ot)
                            for k in ks:
                                nr = CHUNK_ROWS[k]
                                drain(k in (0, 2, 4),
                                      ot[:, 1 + CHUNK_OFF[k]:1 + CHUNK_OFF[k] + nr,
                                         1:49],
                                      qt[:, k % 3, :48 * nr],
                                      btb[:, 0:1], btb[:, 1:2])
                        else:
                            if half == 0:
                                ft = fpool.tile([NROWS, 48, 48], f32,
                                                name=f"f{y}", tag="fo")
                            for k in ks:
                                nr = CHUNK_ROWS[k]
                                drain(k in (0, 2, 4),
                                      ft[:, CHUNK_OFF[k]:CHUNK_OFF[k] + nr, :],
                                      qt[:, k % 3, :48 * nr],
                                      btb[:, 0:1], btb[:, 1:2])
                    if not final:
                        nc.sync.dma_start(dst.ap()[:, y], ot[:])
                    else:
                        nc.sync.dma_start(outd.ap()[:, y - HALO], ft[:])

    nc.compile()
    return nc


def kernel(f, bondary, Wg, bg, W1, b1, W2, b2, Wd, bd):
    from concourse.bass_utils import run_bass_kernel_spmd

    f = np.asarray(f, np.float32)
    bondary = np.asarray(bondary, np.float32)

    if "nc" not in _cached:
        _cached["nc"] = _build_program()
    nc = _cached["nc"]

    w = _build_weights(Wg, bg, W1, b1, W2, b2, Wd, bd)
    in_maps = []
    for core in range(NCORES):
        b, q = core // 4, core % 4
        in_maps.append({
            "fsrc": _make_slab(f[b], q),
            "bndd": _make_slab(bondary[b], q),
            "wAd": w["wA"],
            "wBd": w["wB"],
            "wDd": w["wD"],
            "btd": _bias_tables(bg, b1, b2, bd, W2, q),
        })

    res = run_bass_kernel_spmd(nc, in_maps, core_ids=list(range(NCORES)))

    out = np.zeros((B, C, D1, D2, D3, D4), np.float32)
    for core in range(NCORES):
        b, q = core // 4, core % 4
        arr = res.results[core]["outd"]  # [72, 12, 48, 48]
        sel = arr.reshape(D4, C, SLAB, 48, 48)
        out[b, :, 12 * q:12 * q + 12] = sel.transpose(1, 2, 3, 4, 0)
    return out


# revision 6
# speedup vs baseline: 1.4953x; 1.4953x over previous
"""Trainium2 Bass kernel, v2: big-matmul formulation (72-contraction).

Layout: activations per x1-plane as [72, 50, 50] bf16 tiles; partition row
= 3*x4 + c (x4 in [0,24), c in [0,3)).  x2/x3 zero-padded 48->50 in free
dims; x4 conv edge handled by truncating the band in the weights (no pad
rows, no halo duplication, no fixups).

Each conv tap (d1,d2,d3) = ONE 72x72 matmul per x2-chunk: lhsT[r=3*x4i+ci,
col=3*x4o+co] = K4[co,ci,a,b,c,x4i-x4o+1] (banded over x4, all 24 x4 in one
matmul).  Stage A: 54 passes (2 input groups x 27 taps); stage B: 28 passes
(27 composed W2oW1 taps + Wd residual).  5 x2-chunks {10,10,10,10,8} per
pass for PSUM bank capacity; A uses banks 0-4, B banks 5-7 in two halves.

This targets the serialized tensor-engine regime measured on this part:
per-matmul cost fits ~9.1 ns x M(out cols) weight-load + 0.42 ns x N(free)
stream + ~42 ns; tile_position tiling and column-splitting were measured
and gain nothing here, so fewest-matmuls-with-widest-output wins:
~31.7k matmuls/core (~0.89 us each) vs ~95k in the 30x32-tile scheme.

Sharding: 8 cores = 2 batch x 4 x1-slabs of 12, recompute halo 8 per side
(28 local planes; layer l computes temp1 on [2l+1, 27-2l), out on
[2l+2, 26-2l)).  Layer outputs ping-pong through per-core DRAM in bf16.
"""

import numpy as np
import ml_dtypes

LAYERS = 4
B, C, D1, D2, D3, D4 = 2, 3, 48, 48, 48, 24
NCORES = 8
SLAB = 12
HALO = 8
NPLANES = SLAB + 2 * HALO   # 28
NROWS = 72                  # 3*x4 + c
PW = 50
CHUNK_ROWS = [10, 10, 10, 10, 8]
CHUNK_OFF = [0, 10, 20, 30, 40]
BH = [(0, (0, 1, 2)), (1, (3, 4))]  # stage-B PSUM halves (3 banks)

BF16 = ml_dtypes.bfloat16

PLANE_LIMIT = None

_cached = {}


def _band72(K4):
    """K4: [co, ci(3), 3,3,3, e(3)] -> [27, 72, 72] lhsT per (d1,d2,d3) tap.
    lhsT[tap, 3*x4i+ci, 3*x4o+co] = K4[co, ci, a, b, c, x4i-x4o+1]."""
    out = np.zeros((27, NROWS, NROWS), np.float32)
    x4o = np.arange(D4)
    for e in range(3):
        x4i = x4o + e - 1
        m = (x4i >= 0) & (x4i < D4)
        for pi in range(27):
            a, b_, c_ = pi // 9, (pi // 3) % 3, pi % 3
            for ci in range(3):
                for co in range(3):
                    out[pi, 3 * x4i[m] + ci, 3 * x4o[m] + co] = K4[co, ci, a, b_, c_, e]
    return out


def _build_weights(Wg, bg, W1, b1, W2, b2, Wd, bd):
    Wg = np.asarray(Wg, np.float32)
    W1 = np.asarray(W1, np.float32)
    W2 = np.asarray(W2, np.float32)
    Wd = np.asarray(Wd, np.float32)
    W21 = np.einsum("lome,lmiabc->loiabce", W2[:, :, :, 0, 0, 0, :], W1[..., 0])

    # stored row-major-first for contiguous SBUF DMA: [L, row(128), 2, 27, col]
    # rows 72..127 are zero: K is padded to 128 partitions; zero weight rows
    # null out whatever the padded rhs partitions contain, so pad content
    # only has to be finite (pool slots are pre-zeroed once).
    wA = np.zeros((LAYERS, 128, 2, 27, NROWS), np.float32)
    wB = np.zeros((LAYERS, 128, 27, NROWS), np.float32)
    wD = np.zeros((LAYERS, 128, NROWS), np.float32)
    for l in range(LAYERS):
        wA[l, :NROWS, 0] = _band72(Wg[l, :, 0:3]).transpose(1, 0, 2)
        wA[l, :NROWS, 1] = _band72(Wg[l, :, 3:6]).transpose(1, 0, 2)
        wB[l, :NROWS] = _band72(W21[l]).transpose(1, 0, 2)
        for x4 in range(D4):
            for ci in range(3):
                for co in range(3):
                    wD[l, 3 * x4 + ci, 3 * x4 + co] = Wd[l, co, ci, 0, 0, 0, 0]
    return {
        "wA": wA.astype(BF16),
        "wB": wB.astype(BF16),
        "wD": wD.astype(BF16),
    }


def _bias_tables(bg, b1, b2, bd, W2, q):
    """[L, 2, NPLANES, 72, 2] fp32: col 0 = mask, col 1 = bias.
    Zero on globally-invalid x1 planes (acts as conv zero-pad)."""
    bg = np.asarray(bg, np.float32)
    b1 = np.asarray(b1, np.float32)
    b2 = np.asarray(b2, np.float32)
    bd = np.asarray(bd, np.float32)
    W2 = np.asarray(W2, np.float32)
    tab = np.zeros((LAYERS, 2, NPLANES, NROWS, 2), np.float32)
    for l in range(LAYERS):
        rowA = np.zeros(NROWS, np.float32)
        rowB = np.zeros(NROWS, np.float32)
        for x4 in range(D4):
            for c in range(3):
                r = 3 * x4 + c
                rowA[r] = bg[l, c]
                acc = b2[l, c] + bd[l, c]
                for e in range(3):
                    if 0 <= x4 + e - 1 < D4:
                        acc += float(np.dot(W2[l, c, :, 0, 0, 0, e], b1[l]))
                rowB[r] = acc
        for p in range(NPLANES):
            g = 12 * q - HALO + p
            if 0 <= g < D1:
                tab[l, 0, p, :, 0] = 1.0
                tab[l, 0, p, :, 1] = rowA
                tab[l, 1, p, :, 0] = 1.0
                tab[l, 1, p, :, 1] = rowB
    return tab


def _make_slab(vol, q):
    """vol: [C, D1, D2, D3, D4] fp32 -> [72, NPLANES, 50, 50] bf16 for core q."""
    slab = np.zeros((NROWS, NPLANES, PW, PW), BF16)
    p = np.arange(NPLANES)
    g = 12 * q - HALO + p
    pmask = (g >= 0) & (g < D1)
    ps, gs = p[pmask], g[pmask]
    # row 3*x4+c <- vol[c, g, :, :, x4]: [D4*C, len(ps), 48, 48]
    vt = vol.transpose(4, 0, 1, 2, 3)  # [D4, C, D1, D2, D3]
    slab[:, ps, 1:49, 1:49] = vt[:, :, gs].reshape(
        NROWS, len(ps), 48, 48).astype(BF16)
    return slab


def _build_program():
    import concourse.bass as bass
    import concourse.mybir as mybir
    import concourse.tile as tile
    from concourse import bacc

    f32 = mybir.dt.float32
    bf16 = mybir.dt.bfloat16

    nc = bacc.Bacc("TRN2", target_bir_lowering=False, debug=False,
                   num_devices=NCORES)

    fsrc = nc.dram_tensor("fsrc", [NROWS, NPLANES, PW, PW], bf16, kind="ExternalInput")
    bndd = nc.dram_tensor("bndd", [NROWS, NPLANES, PW, PW], bf16, kind="ExternalInput")
    wAd = nc.dram_tensor("wAd", [LAYERS, 128, 2, 27, NROWS], bf16, kind="ExternalInput")
    wBd = nc.dram_tensor("wBd", [LAYERS, 128, 27, NROWS], bf16, kind="ExternalInput")
    wDd = nc.dram_tensor("wDd", [LAYERS, 128, NROWS], bf16, kind="ExternalInput")
    btd = nc.dram_tensor("btd", [LAYERS, 2, NPLANES, NROWS, 2], f32, kind="ExternalInput")
    bufA = nc.dram_tensor("bufA", [NROWS, NPLANES, PW, PW], bf16, kind="Internal")
    bufB = nc.dram_tensor("bufB", [NROWS, NPLANES, PW, PW], bf16, kind="Internal")
    outd = nc.dram_tensor("outd", [NROWS, SLAB, 48, 48], f32, kind="ExternalOutput")

    with tile.TileContext(nc) as tc:
        with (
            tc.tile_pool(name="wpool", bufs=2) as wpool,
            tc.tile_pool(name="spool", bufs=6) as spool,
            tc.tile_pool(name="bpool", bufs=5) as bpool,
            tc.tile_pool(name="tpool", bufs=4) as tpool,
            tc.tile_pool(name="opool", bufs=3) as opool,
            tc.tile_pool(name="fpool", bufs=2) as fpool,
            tc.tile_pool(name="btpool", bufs=4) as btpool,
            tc.tile_pool(name="ppa", bufs=1, space="PSUM") as ppa,
            tc.tile_pool(name="ppb", bufs=1, space="PSUM") as ppb,
        ):
            def zero_borders(t):
                nc.vector.memset(t[0:NROWS, 0, :], 0.0)
                nc.vector.memset(t[0:NROWS, PW - 1, :], 0.0)
                nc.vector.memset(t[0:NROWS, :, 0], 0.0)
                nc.vector.memset(t[0:NROWS, :, PW - 1], 0.0)

            COPY = mybir.ActivationFunctionType.Identity
            MUL = mybir.AluOpType.mult
            ADD = mybir.AluOpType.add

            def drain(eng_is_act, dst_ap, src_ap, mask_ap, bias_ap):
                if eng_is_act:
                    nc.scalar.activation(dst_ap, src_ap, COPY,
                                         bias=bias_ap, scale=mask_ap)
                else:
                    nc.vector.tensor_scalar(dst_ap, src_ap, mask_ap, bias_ap,
                                            MUL, ADD)

            # Pre-zero partition rows 72..127 of every pooled rhs slot once:
            # later allocations reuse these slots and only ever write rows
            # 0..71, so the pad rows stay zero (zero weights x finite pad
            # rows contribute nothing; this guards against NaN garbage).
            for pool, tag, nslots in ((spool, "sw", 6), (bpool, "bw", 5),
                                      (tpool, "tw", 4)):
                for i in range(nslots):
                    zt = pool.tile([128, PW, PW], bf16, name=f"z{tag}{i}",
                                   tag=tag)
                    # [64:128]: partition ranges must be aligned power-of-two
                    # blocks; rows 64..71 are rewritten by every real load.
                    nc.vector.memset(zt[64:128, :, :], 0.0)

            for l in range(LAYERS):
                src = [fsrc, bufA, bufB, bufA][l]
                dst = [bufA, bufB, bufA, None][l]
                wa = wpool.tile([128, 2, 27, NROWS], bf16, name=f"wa{l}", tag="wa")
                wb = wpool.tile([128, 27, NROWS], bf16, name=f"wb{l}", tag="wb")
                wd = wpool.tile([128, NROWS], bf16, name=f"wd{l}", tag="wd")
                nc.sync.dma_start(wa[:], wAd.ap()[l])
                nc.sync.dma_start(wb[:], wBd.ap()[l])
                nc.sync.dma_start(wd[:], wDd.ap()[l])

                A_lo, A_hi = 2 * l + 1, 27 - 2 * l
                B_lo, B_hi = 2 * l + 2, 26 - 2 * l
                if PLANE_LIMIT is not None:
                    A_lo, A_hi = max(A_lo, PLANE_LIMIT[0]), min(A_hi, PLANE_LIMIT[1])
                    B_lo, B_hi = max(B_lo, A_lo + 1), min(B_hi, A_hi - 1)
                scache, bcache, tcache = {}, {}, {}

                for x in range(A_lo, A_hi):
                    for p in (x - 1, x, x + 1):
                        if p not in scache:
                            st = spool.tile([128, PW, PW], bf16,
                                            name=f"s{l}_{p}", tag="sw")
                            nc.sync.dma_start(st[0:NROWS], src.ap()[:, p])
                            scache[p] = st
                        if p not in bcache:
                            bt_ = bpool.tile([128, PW, PW], bf16,
                                             name=f"b{l}_{p}", tag="bw")
                            nc.sync.dma_start(bt_[0:NROWS], bndd.ap()[:, p])
                            bcache[p] = bt_
                    bta = btpool.tile([NROWS, 2], f32, name=f"bta{l}_{x}", tag="bt")
                    nc.sync.dma_start(bta[:], btd.ap()[l, 0, x])

                    # ---- stage A: temp1 plane x; 54 passes x 5 chunks ----
                    pt = ppa.tile([NROWS, 5, 512], f32, name=f"pa{l}_{x}", tag="pa")
                    for pi in range(54):
                        g, p27 = divmod(pi, 27)
                        a, b_, c_ = p27 // 9, (p27 // 3) % 3, p27 % 3
                        rt = (scache if g == 0 else bcache)[x + a - 1]
                        for k in range(5):
                            nr = CHUNK_ROWS[k]
                            nc.tensor.matmul(
                                pt[:, k, :48 * nr],
                                wa[:, g, p27, :],
                                rt[0:128,
                                   CHUNK_OFF[k] + b_:CHUNK_OFF[k] + b_ + nr,
                                   c_:c_ + 48],
                                start=(pi == 0), stop=(pi == 53),
                                skip_group_check=True,
                            )
                    tt = tpool.tile([128, PW, PW], bf16, name=f"t{l}_{x}", tag="tw")
                    tcache[x] = tt
                    zero_borders(tt)
                    for k in range(5):
                        nr = CHUNK_ROWS[k]
                        drain(k in (0, 2, 4),
                              tt[0:NROWS, 1 + CHUNK_OFF[k]:1 + CHUNK_OFF[k] + nr,
                                 1:49],
                              pt[:, k, :48 * nr],
                              bta[:, 0:1], bta[:, 1:2])

                    # ---- stage B for plane y = x-1 ----
                    y = x - 1
                    if not (B_lo <= y < B_hi):
                        continue
                    final = l == LAYERS - 1
                    if final and not (HALO <= y < HALO + SLAB):
                        continue
                    btb = btpool.tile([NROWS, 2], f32, name=f"btb{l}_{y}", tag="bt")
                    nc.sync.dma_start(btb[:], btd.ap()[l, 1, y])
                    ot = ft = None
                    for half, ks in BH:
                        qt = ppb.tile([NROWS, 3, 512], f32, name=f"pb{l}_{y}_{half}",
                                      tag="pb")
                        for pi in range(28):
                            for k in ks:
                                nr = CHUNK_ROWS[k]
                                kr = k % 3
                                if pi < 27:
                                    a, b_, c_ = pi // 9, (pi // 3) % 3, pi % 3
                                    rt = tcache[y + a - 1]
                                    nc.tensor.matmul(
                                        qt[:, kr, :48 * nr],
                                        wb[:, pi, :],
                                        rt[0:128,
                                           CHUNK_OFF[k] + b_:CHUNK_OFF[k] + b_ + nr,
                                           c_:c_ + 48],
                                        start=(pi == 0), stop=False,
                                        skip_group_check=True,
                                    )
                                else:
                                    rt = scache[y]
                                    nc.tensor.matmul(
                                        qt[:, kr, :48 * nr],
                                        wd[:, :],
                                        rt[0:128,
                                           1 + CHUNK_OFF[k]:1 + CHUNK_OFF[k] + nr,
                                           1:49],
                                        start=False, stop=True,
                                        skip_group_check=True,
                                    )
                        if not final:
                            if half == 0:
                                ot = opool.tile([NROWS, PW, PW], bf16,
                                                name=f"o{l}_{y}", tag="ow")
                                zero_borders(ot)
                            for k in ks:
                                nr = CHUNK_ROWS[k]
                                drain(k in (0, 2, 4),
                                      ot[:, 1 + CHUNK_OFF[k]:1 + CHUNK_OFF[k] + nr,
                                         1:49],
                                      qt[:, k % 3, :48 * nr],
                                      btb[:, 0:1], btb[:, 1:2])
                        else:
                            if half == 0:
                                ft = fpool.tile([NROWS, 48, 48], f32,
                                                name=f"f{y}", tag="fo")
                            for k in ks:
                                nr = CHUNK_ROWS[k]
                                drain(k in (0, 2, 4),
                                      ft[:, CHUNK_OFF[k]:CHUNK_OFF[k] + nr, :],
                                      qt[:, k % 3, :48 * nr],
                                      btb[:, 0:1], btb[:, 1:2])
                    if not final:
                        nc.sync.dma_start(dst.ap()[:, y], ot[:])
                    else:
                        nc.sync.dma_start(outd.ap()[:, y - HALO], ft[:])

    nc.compile()
    return nc


def kernel(f, bondary, Wg, bg, W1, b1, W2, b2, Wd, bd):
    from concourse.bass_utils import run_bass_kernel_spmd

    f = np.asarray(f, np.float32)
    bondary = np.asarray(bondary, np.float32)

    if "nc" not in _cached:
        _cached["nc"] = _build_program()
    nc = _cached["nc"]

    w = _build_weights(Wg, bg, W1, b1, W2, b2, Wd, bd)
    in_maps = []
    for core in range(NCORES):
        b, q = core // 4, core % 4
        in_maps.append({
            "fsrc": _make_slab(f[b], q),
            "bndd": _make_slab(bondary[b], q),
            "wAd": w["wA"],
            "wBd": w["wB"],
            "wDd": w["wD"],
            "btd": _bias_tables(bg, b1, b2, bd, W2, q),
        })

    res = run_bass_kernel_spmd(nc, in_maps, core_ids=list(range(NCORES)))

    out = np.zeros((B, C, D1, D2, D3, D4), np.float32)
    for core in range(NCORES):
        b, q = core // 4, core % 4
        arr = np.asarray(res.results[core]["outd"], np.float32)
        sel = arr.reshape(D4, C, SLAB, 48, 48)
        out[b, :, 12 * q:12 * q + 12] = sel.transpose(1, 2, 3, 4, 0)
    return out
